# revision 1
# baseline (speedup 1.0000x reference)
"""Distributed Taylor-series diffusion kernel for Trainium2 (8 NeuronCores).

Computes out[:, c] = expm(-t[c] * L) @ x[:, c] via a truncated Taylor series
    y = sum_{k=0}^{K} (-t)^k L^k x / k!
with K = 8 (remainder ~7e-9, far below the ~4e-5 float32r matmul noise and
the fp32 noise of the order-25 reference).

Distribution: L is symmetric, so core j holds the column block
L[:, 768j:768(j+1)] resident in SBUF (18.9 MB) and computes the transposed
shard z_T[c, v] = (z.T @ Lblk)[c, v] of each unscaled power z_k = L^k x.
The per-channel Taylor coefficients c_k = (-t_c)^k / k! are folded into the
accumulation (scaling commutes with L). Each step's shard is produced in two
v-halves: as soon as half 1's matmuls stop, it is block-transposed (DVE,
cross-partition) to natural [v, c] layout and its 24 KB all-gather launches
while half 2's matmuls still run — hiding most of the collective latency.
Matmuls run in float32r mode (fp32 storage, ~1.5e-4 matmul relative error,
4x plain-fp32 speed).
"""

import os
import sys

sys.path.insert(0, "/opt/trn_rl_repo")

import numpy as np

import concourse.bass as bass
import concourse.mybir as mybir
import concourse.tile as tile
from concourse import bacc
from concourse.bass_utils import run_bass_kernel_spmd

F32 = mybir.dt.float32
F32R = mybir.dt.float32r

V = 6144
C = 16
N_CORES = 8
VS = V // N_CORES          # 768 columns of L per core
NUT = V // 128             # 48 u-tiles (contraction dim)
LOCT = VS // 128           # 6 u-tiles produced per core per step
HV = VS // 2               # 384: v-half per core
K_STEPS = 8

TRACE = False
LAST_RESULT = None

_cached_nc = None


def _build():
    nc = bacc.Bacc("TRN2", target_bir_lowering=False, debug=False,
                   num_devices=N_CORES)

    L_in = nc.dram_tensor("L", [V, VS], F32R, kind="ExternalInput")
    x_in = nc.dram_tensor("x", [V, C], F32R, kind="ExternalInput")
    ts_in = nc.dram_tensor("ts", [K_STEPS, C], F32, kind="ExternalInput")
    out_d = nc.dram_tensor("out", [C, VS], F32, kind="ExternalOutput")

    rg = [list(range(N_CORES))]

    with tile.TileContext(nc) as tc:
        with (
            tc.tile_pool(name="Lp", bufs=1) as Lp,
            tc.tile_pool(name="natp", bufs=2) as natp,
            tc.tile_pool(name="stgp", bufs=2) as stgp,
            tc.tile_pool(name="accp", bufs=1) as accp,
            tc.tile_pool(name="tsp", bufs=1) as tsp,
            tc.tile_pool(name="psp", bufs=2, space="PSUM") as psp,
            tc.tile_pool(name="dram", bufs=2, space="DRAM") as dram,
        ):
            # ---- Taylor coefficients: ts_sb[c, k] = (-t_c)^(k+1) / (k+1)!
            ts_sb = tsp.tile([C, K_STEPS], F32)
            nc.sync.dma_start(ts_sb[:], ts_in[:].rearrange("k c -> c k"))

            # ---- z_0 = x (natural layout); loaded before L so step 1 can
            # start as soon as the first L tiles land
            def new_nat():
                # natural-layout power z_k: 8 rank blocks of [128, 6*32]
                # (16 valid cols per 32-col group)
                return [natp.tile([128, LOCT * 32], F32R, tag=f"nat{r}",
                                  name=f"nat{r}")
                        for r in range(N_CORES)]

            nat = new_nat()
            for r in range(N_CORES):
                eng = nc.sync if r % 2 == 0 else nc.scalar
                eng.dma_start(
                    nat[r][:].rearrange("p (i e) -> p i e", e=32)[:, :, 0:C],
                    x_in[VS * r:VS * (r + 1), :].rearrange(
                        "(i p) c -> p i c", p=128),
                )

            # ---- warm up the collective path with a tiny AllGather that
            # runs concurrently with the L load
            w_in = dram.tile([2, C], F32, tag="warm_in")
            w_out = dram.tile([2 * N_CORES, C], F32, tag="warm_out",
                              addr_space="Shared")
            nc.sync.dma_start(w_in[:], ts_in[0:2, :])
            nc.gpsimd.collective_compute(
                "AllGather", mybir.AluOpType.bypass, replica_groups=rg,
                ins=[w_in.opt()], outs=[w_out.opt()],
            )

            # ---- resident L: 48 tiles of [128, 768]
            Lt = []
            for u in range(NUT):
                lt = Lp.tile([128, VS], F32R, tag=f"L{u}", name=f"L{u}")
                nc.sync.dma_start(lt[:], L_in[128 * u:128 * (u + 1), :])
                Lt.append(lt)

            # ---- accumulator (transposed shard), partitions 0:16 valid
            acc = accp.tile([32, VS], F32)
            nc.vector.memset(acc[:], 0.0)

            # u-tile order: for each rank its first-half tiles (i < 3) come
            # first, so after the split all-gather the next step can start
            # on half-1 weights while half 2 is still in flight.
            u_order = [6 * r + i for i in range(LOCT) for r in range(N_CORES)]

            def half_matmuls(ps, h, k):
                lo = HV * h
                for idx, u in enumerate(u_order):
                    lhsT = nat[u // LOCT][:, (u % LOCT) * 32:
                                          (u % LOCT) * 32 + C]
                    nc.tensor.matmul(ps[0:C, :], lhsT, Lt[u][:, lo:lo + HV],
                                     start=(idx == 0), stop=(idx == NUT - 1))

            for k in range(1, K_STEPS + 1):
                pss = [psp.tile([32, HV], F32, tag=f"ps{h}", name=f"ps{h}")
                       for h in range(2)]
                for h in (0, 1):
                    half_matmuls(pss[h], h, k)

                    if k < K_STEPS:
                        # block-transpose this half to natural layout:
                        # v-local = HV*h + 32kk + r2 -> stg partition
                        # 32*(kk%4)+r2, col 32*(3h + kk//4) + c
                        stg = stgp.tile([128, LOCT // 2 * 32], F32R,
                                        tag=f"stg{h}", name=f"stg{h}")
                        ps_blocks = pss[h][:].rearrange(
                            "p (kk e) -> p kk e", e=32)
                        for b in range(4):
                            nc.vector.transpose(
                                stg[32 * b:32 * (b + 1), :].bitcast(F32)
                                .rearrange("p (kk e) -> p kk e", e=32),
                                ps_blocks[:, b::4, :],
                            )
                        b_in = dram.tile([HV, C], F32R, tag=f"bin{h}",
                                         name=f"bin{h}")
                        b_out = dram.tile([N_CORES * HV, C], F32R,
                                          tag=f"bout{h}", name=f"bout{h}",
                                          addr_space="Shared")
                        nc.sync.dma_start(
                            b_in[:].rearrange("(i p) c -> p i c", p=128),
                            stg[:].rearrange("p (i e) -> p i e",
                                             e=32)[:, :, 0:C],
                        )
                        nc.gpsimd.collective_compute(
                            "AllGather", mybir.AluOpType.bypass,
                            replica_groups=rg,
                            ins=[b_in.opt()], outs=[b_out.opt()],
                        )
                        if h == 0:
                            nat_next = new_nat()
                        for r in range(N_CORES):
                            eng = nc.sync if r % 2 == 0 else nc.scalar
                            eng.dma_start(
                                nat_next[r][:].rearrange(
                                    "p (i e) -> p i e", e=32
                                )[:, 3 * h:3 * h + 3, 0:C],
                                b_out[HV * r:HV * (r + 1), :].rearrange(
                                    "(i p) c -> p i c", p=128),
                            )

                    # acc += c_k * z_k for this half
                    nc.vector.scalar_tensor_tensor(
                        acc[0:C, HV * h:HV * (h + 1)], pss[h][0:C, :],
                        ts_sb[:, k - 1:k], acc[0:C, HV * h:HV * (h + 1)],
                        op0=mybir.AluOpType.mult, op1=mybir.AluOpType.add,
                    )
                if k < K_STEPS:
                    nat = nat_next

            nc.sync.dma_start(out_d[:], acc[0:C, :])

    nc.compile()
    return nc


def _get_nc():
    global _cached_nc
    if _cached_nc is None:
        _cached_nc = _build()
    return _cached_nc


def kernel(x: np.ndarray, L: np.ndarray, t: np.ndarray) -> np.ndarray:
    global LAST_RESULT
    x = np.ascontiguousarray(np.asarray(x, dtype=np.float32))
    L = np.asarray(L, dtype=np.float32)
    t = np.asarray(t, dtype=np.float32)
    assert x.shape == (V, C) and L.shape == (V, V) and t.shape == (C,)

    # c_k = (-t)^k / k!, computed the way the reference's recurrence rounds:
    # c_k = c_{k-1} * (-t / k), in float32.
    tc_ = np.clip(t, 1e-8, None)
    cs = []
    cur = np.ones(C, np.float32)
    for k in range(1, K_STEPS + 1):
        cur = cur * (-tc_ / np.float32(k))
        cs.append(cur)
    ts = np.ascontiguousarray(np.stack(cs).astype(np.float32))

    in_maps = []
    for j in range(N_CORES):
        in_maps.append({
            "L": np.ascontiguousarray(L[:, VS * j:VS * (j + 1)]),
            "x": x,
            "ts": ts,
        })

    nc = _get_nc()
    res = run_bass_kernel_spmd(nc, in_maps, core_ids=list(range(N_CORES)),
                               trace=TRACE)
    LAST_RESULT = res

    y = np.empty((V, C), dtype=np.float32)
    for j in range(N_CORES):
        y[VS * j:VS * (j + 1), :] = res.results[j]["out"].T
    return x + y



# revision 2
# speedup vs baseline: 1.9872x; 1.9872x over previous
"""Distributed Taylor-series diffusion kernel for Trainium2 (8 NeuronCores).

Computes out[:, c] = expm(-t[c] * L) @ x[:, c] via a truncated Taylor series
    y = sum_{k=0}^{K} (-t)^k L^k x / k!
with K = 4. Writing L = I - S (S symmetric, ||S|| ~ 0.8, entries ~5e-3), each
power step is z_{k+1} = z_k - S z_k: the identity part stays in fp32, so only
the small S-product runs in fp8 e4m3 (S pre-scaled x32 on host, rescaled in
the accumulation). Measured rel err vs the order-25 fp32 reference ~4e-3,
well under the 2e-2 gate.

Distribution: S is symmetric; core j holds the column block S[:, 768j:768j+768]
resident in SBUF as fp8 (4.7 MB), pre-permuted on host into 24 DoubleRow
contraction tiles [128, 2, 768] (256 contraction rows per matmul instruction,
2 rows/cycle -> ~4us of PE time per step). Each step core j computes the
transposed shard (S z)^T[:, block_j] = z^T S[:, block_j] in two 384-column
PSUM halves; each half is written back as z^T_next = z^T_prev - psum/32,
all-gathered ([16,384] fp32, 16 descriptors of 1.5KB), DMA'd back as one
[128, 384] gathered tile (8 split DMAs across queues), block-transposed to
natural layout on the DVE and converted to e4m3 for the next step's lhsT.
The per-channel Taylor coefficients are folded into a per-half fp32
accumulator; the k=0 term (x itself) is added exactly on the host.
"""

import sys

sys.path.insert(0, "/opt/trn_rl_repo")

import numpy as np
import ml_dtypes

import concourse.bass as bass
import concourse.mybir as mybir
import concourse.tile as tile
from concourse import bacc
from concourse.bass_utils import run_bass_kernel_spmd

F32 = mybir.dt.float32
FP8 = mybir.dt.float8e4

V = 6144
C = 16
N_CORES = 8
VS = V // N_CORES          # 768 columns of S per core
HV = VS // 2               # 384: output half width
K_STEPS = 4
NDT = 24                   # DoubleRow contraction tiles (256 rows each)
SSCALE = 32.0              # fp8 quantization scale for S

TRACE = False
LAST_RESULT = None

_cached_nc = None
_cached_prep = None


def _v_index(d, P, i):
    """Global row v held by (dtile d, partition P, slot i)."""
    a = P // 32
    e = P % 32
    h = d // 12
    m = (d // 4) % 3
    q = d % 4
    return 768 * (2 * a + i) + 384 * h + 128 * m + 32 * q + e


def _build():
    nc = bacc.Bacc("TRN2", target_bir_lowering=False, debug=False,
                   num_devices=N_CORES)

    S_in = nc.dram_tensor("S8", [NDT, 128, 2 * VS], FP8, kind="ExternalInput")
    x8_in = nc.dram_tensor("x8", [128, NDT * 32], FP8, kind="ExternalInput")
    xt_in = nc.dram_tensor("xt", [C, VS], F32, kind="ExternalInput")
    ts_in = nc.dram_tensor("ts", [K_STEPS, C], F32, kind="ExternalInput")
    out_d = nc.dram_tensor("out", [C, VS], F32, kind="ExternalOutput")

    rg = [list(range(N_CORES))]

    with tile.TileContext(nc) as tc:
        with (
            tc.tile_pool(name="Sp", bufs=1) as Sp,
            tc.tile_pool(name="natp", bufs=2) as natp,
            tc.tile_pool(name="natfp", bufs=2) as natfp,
            tc.tile_pool(name="ztp", bufs=2) as ztp,
            tc.tile_pool(name="znp", bufs=2) as znp,
            tc.tile_pool(name="smallp", bufs=1) as smallp,
            tc.tile_pool(name="psp", bufs=2, space="PSUM") as psp,
            tc.tile_pool(name="dram", bufs=2, space="DRAM") as dram,
        ):
            # ---- Taylor coefficients ts_sb[c, k] = (-t_c)^(k+1)/(k+1)!
            ts_sb = smallp.tile([C, K_STEPS], F32)
            nc.sync.dma_start(ts_sb[:], ts_in[:].rearrange("k c -> c k"))

            # ---- warm up the collective path ASAP (first collective pays a
            # large one-time setup; run it concurrently with the S load)
            w_in = dram.tile([2, C], F32, tag="warm_in")
            w_out = dram.tile([2 * N_CORES, C], F32, tag="warm_out",
                              addr_space="Shared")
            nc.sync.dma_start(w_in[:], ts_in[0:2, :])
            nc.gpsimd.collective_compute(
                "AllGather", mybir.AluOpType.bypass, replica_groups=rg,
                ins=[w_in.opt()], outs=[w_out.opt()],
            )

            # ---- z_0 = x in natural fp8 layout (host-prepped, full V)
            nat = natp.tile([128, NDT * 32], FP8, tag="nat", name="nat1")
            nc.sync.dma_start(nat[:], x8_in[:])

            # ---- own-shard x^T (fp32) for the step-1 identity part
            xt_sb = smallp.tile([C, VS], F32)
            nc.scalar.dma_start(xt_sb[:], xt_in[:])

            # ---- accumulator acc[c, v_local] = sum_k c_k z_k^T
            acc = smallp.tile([C, VS], F32)
            nc.vector.memset(acc[:], 0.0)

            # ---- resident S in fp8: 24 DoubleRow tiles [128, 2, 768]
            St = []
            for d in range(NDT):
                st = Sp.tile([128, 2 * VS], FP8, tag=f"S{d}", name=f"S{d}")
                eng = nc.sync if d % 2 == 0 else nc.scalar
                eng.dma_start(st[:], S_in[d, :, :])
                St.append(st)

            def lhsT_view(nt, d):
                # [128, 2, 16] DoubleRow stationary slice of the nat tile
                return nt[:].rearrange("p (d i c) -> p d i c",
                                       d=NDT, i=2, c=C)[:, d, :, :]

            zn_prev = None
            for k in range(1, K_STEPS + 1):
                pss = [psp.tile([32, HV], F32, tag=f"ps{g}", name=f"ps{g}_{k}")
                       for g in range(2)]
                # emission order g0h0, g1h0, g0h1, g1h1: the tensor engine can
                # start on input half h=0 while h=1's all-gather is in flight,
                # and psum g0 stops ~1us before g1 so its chain launches early
                for h in (0, 1):
                    for g in (0, 1):
                        for dd in range(12):
                            d = 12 * h + dd
                            idx = 12 * h + dd
                            nc.tensor.matmul(
                                pss[g][0:C, :],
                                lhsT_view(nat, d),
                                St[d][:].rearrange("p (i n) -> p i n",
                                                   i=2)[:, :, HV * g:
                                                        HV * (g + 1)],
                                start=(idx == 0), stop=(idx == NDT - 1),
                                perf_mode=mybir.MatmulPerfMode.DoubleRow,
                            )

                zn = [znp.tile([C, HV], F32, tag=f"zn{g}", name=f"zn{g}_{k}")
                      for g in range(2)]
                if k < K_STEPS:
                    nat_next = natp.tile([128, NDT * 32], FP8, tag="nat",
                                         name=f"nat{k + 1}")
                for g in (0, 1):
                    zprev = (xt_sb[:, HV * g:HV * (g + 1)] if k == 1
                             else zn_prev[g][:, :])
                    # z_next^T = z_prev^T - psum/SSCALE
                    nc.vector.scalar_tensor_tensor(
                        zn[g][:, :], pss[g][0:C, :], -1.0 / SSCALE, zprev,
                        op0=mybir.AluOpType.mult, op1=mybir.AluOpType.add,
                    )
                    # acc += c_k * z_next^T
                    nc.vector.scalar_tensor_tensor(
                        acc[:, HV * g:HV * (g + 1)], zn[g][:, :],
                        ts_sb[:, k - 1:k], acc[:, HV * g:HV * (g + 1)],
                        op0=mybir.AluOpType.mult, op1=mybir.AluOpType.add,
                    )
                    if k < K_STEPS:
                        b_in = dram.tile([C, HV], F32, tag=f"bin{g}",
                                         name=f"bin{g}_{k}")
                        b_out = dram.tile([N_CORES * C, HV], F32,
                                          tag=f"bout{g}", name=f"bout{g}_{k}",
                                          addr_space="Shared")
                        nc.sync.dma_start(b_in[:], zn[g][:, :])
                        nc.gpsimd.collective_compute(
                            "AllGather", mybir.AluOpType.bypass,
                            replica_groups=rg,
                            ins=[b_in.opt()], outs=[b_out.opt()],
                        )
                        # gathered z^T: partition P = 16r + c, col = v offset
                        zt = ztp.tile([128, HV], F32, tag=f"zt{g}",
                                      name=f"zt{g}_{k}")
                        for s in range(N_CORES):
                            eng = nc.scalar if s % 2 == 0 else nc.sync
                            eng.dma_start(zt[C * s:C * (s + 1), :],
                                          b_out[C * s:C * (s + 1), :])
                        # block-transpose to natural layout (fp32)
                        natf = natfp.tile([128, HV], F32, tag=f"natf{g}",
                                          name=f"natf{g}_{k}")
                        ztv = zt[:].rearrange("p (m q e) -> p m q e",
                                              m=3, q=4, e=32)
                        nfv = natf[:].rearrange("p (m q w) -> p m q w",
                                                m=3, q=4, w=32)
                        for q in range(4):
                            nc.vector.transpose(nfv[:, :, q, :],
                                                ztv[:, :, q, :])
                        # convert to fp8 into half h=g of the next nat tile
                        nc.scalar.copy(
                            nat_next[:, 12 * 32 * g:12 * 32 * (g + 1)],
                            natf[:],
                        )
                zn_prev = zn
                if k < K_STEPS:
                    nat = nat_next

            nc.sync.dma_start(out_d[:], acc[:])

    nc.compile()
    return nc


def _get_nc():
    global _cached_nc
    if _cached_nc is None:
        _cached_nc = _build()
    return _cached_nc


def _host_prep(x, L, t):
    """Permute/quantize inputs into the kernel's layouts."""
    e4 = ml_dtypes.float8_e4m3

    # Taylor coefficients with the reference's rounding: c_k = c_{k-1}*(-t/k)
    tc_ = np.clip(t, 1e-8, None)
    cs = []
    cur = np.ones(C, np.float32)
    for k in range(1, K_STEPS + 1):
        cur = cur * (-tc_ / np.float32(k))
        cs.append(cur)
    ts = np.ascontiguousarray(np.stack(cs).astype(np.float32))

    # v index for (dtile, partition, slot)
    dd, PP, ii = np.meshgrid(np.arange(NDT), np.arange(128), np.arange(2),
                             indexing="ij")
    vidx = _v_index(dd, PP, ii)              # [24, 128, 2]

    # natural-layout fp8 x: x8[P, d*32 + i*16 + c] = x[v(d,P,i), c]
    xq = x.astype(e4)
    x8 = xq[vidx.transpose(1, 0, 2)].reshape(128, NDT * 32)
    x8 = np.ascontiguousarray(x8)

    xt = np.ascontiguousarray(x.T)           # [C, V] fp32

    in_maps = []
    for j in range(N_CORES):
        Sblk = -L[:, VS * j:VS * (j + 1)] * np.float32(SSCALE)
        idx = np.arange(VS)
        Sblk[VS * j + idx, idx] += np.float32(SSCALE)
        Sq = Sblk.astype(e4)                 # [V, 768] fp8
        S8 = Sq[vidx].reshape(NDT, 128, 2 * VS)
        in_maps.append({
            "S8": np.ascontiguousarray(S8),
            "x8": x8,
            "xt": np.ascontiguousarray(xt[:, VS * j:VS * (j + 1)]),
            "ts": ts,
        })
    return in_maps


def kernel(x: np.ndarray, L: np.ndarray, t: np.ndarray) -> np.ndarray:
    global LAST_RESULT
    x = np.ascontiguousarray(np.asarray(x, dtype=np.float32))
    L = np.asarray(L, dtype=np.float32)
    t = np.asarray(t, dtype=np.float32)
    assert x.shape == (V, C) and L.shape == (V, V) and t.shape == (C,)

    in_maps = _host_prep(x, L, t)
    nc = _get_nc()
    res = run_bass_kernel_spmd(nc, in_maps, core_ids=list(range(N_CORES)),
                               trace=TRACE)
    LAST_RESULT = res

    y = np.empty((V, C), dtype=np.float32)
    for j in range(N_CORES):
        y[VS * j:VS * (j + 1), :] = res.results[j]["out"].T
    return x + y


# revision 3
# speedup vs baseline: 2.2631x; 1.1388x over previous
"""Distributed Taylor-series diffusion kernel for Trainium2 (8 NeuronCores).

Computes out[:, c] = expm(-t[c] * L) @ x[:, c] via a truncated Taylor series
    y = sum_{k=0}^{K} (-t)^k L^k x / k!
with K = 3. Writing L = I - S (S symmetric, ||S|| ~ 0.8, entries ~5e-3), each
power step is z_{k+1} = z_k - S z_k: the identity part stays in fp32, so only
the small S-product runs in fp8 e4m3 (S pre-scaled x32 on host, rescaled in
the accumulation). Measured rel err vs the order-25 fp32 reference ~3e-3,
well under the 2e-2 gate (fp8 quantization, not series truncation, dominates).

Distribution: S is symmetric; core j holds the column block S[:, 768j:768j+768]
resident in SBUF as fp8 (4.7 MB), pre-permuted on host into 24 DoubleRow
contraction tiles [128, 2, 768] (256 contraction rows per matmul instruction,
2 rows/cycle). Each step core j computes the transposed shard
(S z)^T[:, block_j] = z^T S[:, block_j] in two 384-column PSUM halves, writes
z^T_next = z^T_prev - psum/32, and one full-width [16, 768] fp32 AllGather per
step (two half-AGs were measured to serialize on the CC engine; one wide AG is
cheaper). The gathered [128, 768] z^T is DMA'd back in 8 partition-splits,
block-transposed to natural layout on the DVE and converted to e4m3 for the
next step's stationary operand. Taylor coefficients are folded into a per-half
fp32 accumulator; the k=0 term (x itself) is added exactly on the host, and
the last step's accumulation is pre-folded so only one vector op and the
output DMA sit on the tail.
"""

import sys

sys.path.insert(0, "/opt/trn_rl_repo")

import numpy as np
import ml_dtypes

import concourse.bass as bass
import concourse.mybir as mybir
import concourse.tile as tile
from concourse import bacc
from concourse.bass_utils import run_bass_kernel_spmd

F32 = mybir.dt.float32
FP8 = mybir.dt.float8e4

V = 6144
C = 16
N_CORES = 8
VS = V // N_CORES          # 768 columns of S per core
HV = VS // 2               # 384: psum half width
K_STEPS = 3
NDT = 24                   # DoubleRow contraction tiles (256 rows each)
SSCALE = 32.0              # fp8 quantization scale for S
NTS = K_STEPS + 1          # coefficient columns (last = -c_K/SSCALE)

TRACE = False
LAST_RESULT = None

_cached_nc = None


def _v_index(d, P, i):
    """Global row v held by (dtile d, partition P, slot i)."""
    a = P // 32
    e = P % 32
    h = d // 12
    m = (d // 4) % 3
    q = d % 4
    return 768 * (2 * a + i) + 384 * h + 128 * m + 32 * q + e


def _build():
    nc = bacc.Bacc("TRN2", target_bir_lowering=False, debug=False,
                   num_devices=N_CORES)

    S_in = nc.dram_tensor("S8", [NDT, 128, 2 * VS], FP8, kind="ExternalInput")
    x8_in = nc.dram_tensor("x8", [128, NDT * 32], FP8, kind="ExternalInput")
    xt_in = nc.dram_tensor("xt", [C, VS], F32, kind="ExternalInput")
    ts_in = nc.dram_tensor("ts", [NTS, C], F32, kind="ExternalInput")
    out_d = nc.dram_tensor("out", [C, VS], F32, kind="ExternalOutput")

    rg = [list(range(N_CORES))]

    with tile.TileContext(nc) as tc:
        with (
            tc.tile_pool(name="Sp", bufs=1) as Sp,
            tc.tile_pool(name="natp", bufs=2) as natp,
            tc.tile_pool(name="natfp", bufs=2) as natfp,
            tc.tile_pool(name="ztp", bufs=2) as ztp,
            tc.tile_pool(name="znp", bufs=2) as znp,
            tc.tile_pool(name="smallp", bufs=1) as smallp,
            tc.tile_pool(name="psp", bufs=2, space="PSUM") as psp,
            tc.tile_pool(name="dram", bufs=2, space="DRAM") as dram,
        ):
            # ---- coefficients ts_sb[c, k]; col K_STEPS holds -c_K/SSCALE
            ts_sb = smallp.tile([C, NTS], F32)
            nc.sync.dma_start(ts_sb[:], ts_in[:].rearrange("k c -> c k"))

            # ---- warm up the collective path ASAP (first collective pays a
            # large one-time setup; run it concurrently with the S load)
            w_in = dram.tile([2, C], F32, tag="warm_in")
            w_out = dram.tile([2 * N_CORES, C], F32, tag="warm_out",
                              addr_space="Shared")
            nc.sync.dma_start(w_in[:], ts_in[0:2, :])
            nc.gpsimd.collective_compute(
                "AllGather", mybir.AluOpType.bypass, replica_groups=rg,
                ins=[w_in.opt()], outs=[w_out.opt()],
            )

            # ---- z_0 = x in natural fp8 layout (host-prepped, full V)
            nat = natp.tile([128, NDT * 32], FP8, tag="nat", name="nat1")
            nc.sync.dma_start(nat[:], x8_in[:])

            # ---- own-shard x^T (fp32) for the step-1 identity part
            xt_sb = smallp.tile([C, VS], F32)
            nc.scalar.dma_start(xt_sb[:], xt_in[:])

            # ---- accumulators (acc through k=K-1; accp pre-folds c_K z_{K-1})
            acc = smallp.tile([C, VS], F32)
            nc.vector.memset(acc[:], 0.0)
            accp = smallp.tile([C, VS], F32)

            # ---- resident S in fp8: 24 DoubleRow tiles [128, 2, 768]
            St = []
            for d in range(NDT):
                st = Sp.tile([128, 2 * VS], FP8, tag=f"S{d}", name=f"S{d}")
                eng = nc.sync if d % 2 == 0 else nc.scalar
                eng.dma_start(st[:], S_in[d, :, :])
                St.append(st)

            def lhsT_view(nt, d):
                # [128, 2, 16] DoubleRow stationary slice of the nat tile
                return nt[:].rearrange("p (d i c) -> p d i c",
                                       d=NDT, i=2, c=C)[:, d, :, :]

            zn_prev = None
            for k in range(1, K_STEPS + 1):
                pss = [psp.tile([32, HV], F32, tag=f"ps{g}", name=f"ps{g}_{k}")
                       for g in range(2)]
                # 48 DoubleRow matmuls: input half h=0 tiles first so the
                # engine can start before the full gathered z is transposed
                for h in (0, 1):
                    for g in (0, 1):
                        for dd in range(12):
                            d = 12 * h + dd
                            idx = 12 * h + dd
                            nc.tensor.matmul(
                                pss[g][0:C, :],
                                lhsT_view(nat, d),
                                St[d][:].rearrange("p (i n) -> p i n",
                                                   i=2)[:, :, HV * g:
                                                        HV * (g + 1)],
                                start=(idx == 0), stop=(idx == NDT - 1),
                                perf_mode=mybir.MatmulPerfMode.DoubleRow,
                            )

                if k < K_STEPS:
                    zn = [znp.tile([C, HV], F32, tag=f"zn{g}",
                                   name=f"zn{g}_{k}") for g in range(2)]
                    nat_next = natp.tile([128, NDT * 32], FP8, tag="nat",
                                         name=f"nat{k + 1}")
                    b_in = dram.tile([C, VS], F32, tag="bin", name=f"bin{k}")
                    b_out = dram.tile([N_CORES * C, VS], F32, tag="bout",
                                      name=f"bout{k}", addr_space="Shared")
                    # z_next^T = z_prev^T - psum/SSCALE (critical path)
                    for g in (0, 1):
                        zprev = (xt_sb[:, HV * g:HV * (g + 1)] if k == 1
                                 else zn_prev[g][:, :])
                        nc.vector.scalar_tensor_tensor(
                            zn[g][:, :], pss[g][0:C, :], -1.0 / SSCALE, zprev,
                            op0=mybir.AluOpType.mult, op1=mybir.AluOpType.add,
                        )
                    for g in (0, 1):
                        eng = nc.sync if g == 0 else nc.scalar
                        eng.dma_start(b_in[:, HV * g:HV * (g + 1)],
                                      zn[g][:, :])
                    nc.gpsimd.collective_compute(
                        "AllGather", mybir.AluOpType.bypass, replica_groups=rg,
                        ins=[b_in.opt()], outs=[b_out.opt()],
                    )
                    # accumulate while the collective is in flight
                    for g in (0, 1):
                        nc.vector.scalar_tensor_tensor(
                            acc[:, HV * g:HV * (g + 1)], zn[g][:, :],
                            ts_sb[:, k - 1:k], acc[:, HV * g:HV * (g + 1)],
                            op0=mybir.AluOpType.mult, op1=mybir.AluOpType.add,
                        )
                    if k == K_STEPS - 1:
                        for g in (0, 1):
                            nc.vector.scalar_tensor_tensor(
                                accp[:, HV * g:HV * (g + 1)], zn[g][:, :],
                                ts_sb[:, k:k + 1], acc[:, HV * g:HV * (g + 1)],
                                op0=mybir.AluOpType.mult,
                                op1=mybir.AluOpType.add,
                            )
                    # gathered z^T [128 = 16r+c, 768], 8 partition-splits
                    zt = ztp.tile([128, VS], F32, tag="zt", name=f"zt{k}")
                    for s in range(N_CORES):
                        eng = nc.scalar if s % 2 == 0 else nc.sync
                        eng.dma_start(zt[C * s:C * (s + 1), :],
                                      b_out[C * s:C * (s + 1), :])
                    # block-transpose to natural layout (fp32), then fp8
                    natf = natfp.tile([128, VS], F32, tag="natf",
                                      name=f"natf{k}")
                    ztv = zt[:].rearrange("p (h m q e) -> p h m q e",
                                          h=2, m=3, q=4, e=32)
                    nfv = natf[:].rearrange("p (h m q w) -> p h m q w",
                                            h=2, m=3, q=4, w=32)
                    for h in (0, 1):
                        for q in range(4):
                            nc.vector.transpose(nfv[:, h, :, q, :],
                                                ztv[:, h, :, q, :])
                        nc.scalar.copy(
                            nat_next[:, 12 * 32 * h:12 * 32 * (h + 1)],
                            natf[:, 12 * 32 * h:12 * 32 * (h + 1)],
                        )
                    zn_prev = zn
                    nat = nat_next
                else:
                    # final step: out = accp - (c_K/SSCALE) psum, then DMA out
                    fin = [znp.tile([C, HV], F32, tag=f"zn{g}",
                                    name=f"fin{g}") for g in range(2)]
                    for g in (0, 1):
                        nc.vector.scalar_tensor_tensor(
                            fin[g][:, :], pss[g][0:C, :],
                            ts_sb[:, K_STEPS:K_STEPS + 1],
                            accp[:, HV * g:HV * (g + 1)],
                            op0=mybir.AluOpType.mult, op1=mybir.AluOpType.add,
                        )
                        eng = nc.sync if g == 0 else nc.scalar
                        eng.dma_start(out_d[:, HV * g:HV * (g + 1)],
                                      fin[g][:, :])

    nc.compile()
    return nc


def _get_nc():
    global _cached_nc
    if _cached_nc is None:
        _cached_nc = _build()
    return _cached_nc


def _host_prep(x, L, t):
    """Permute/quantize inputs into the kernel's layouts."""
    e4 = ml_dtypes.float8_e4m3

    # Taylor coefficients with the reference's rounding: c_k = c_{k-1}*(-t/k)
    tc_ = np.clip(t, 1e-8, None)
    cs = []
    cur = np.ones(C, np.float32)
    for k in range(1, K_STEPS + 1):
        cur = cur * (-tc_ / np.float32(k))
        cs.append(cur)
    cs.append(-cs[-1] / np.float32(SSCALE))   # aux col for the final fold
    ts = np.ascontiguousarray(np.stack(cs).astype(np.float32))

    # v index for (dtile, partition, slot)
    dd, PP, ii = np.meshgrid(np.arange(NDT), np.arange(128), np.arange(2),
                             indexing="ij")
    vidx = _v_index(dd, PP, ii)              # [24, 128, 2]

    # natural-layout fp8 x: x8[P, d*32 + i*16 + c] = x[v(d,P,i), c]
    xq = x.astype(e4)
    x8 = xq[vidx.transpose(1, 0, 2)].reshape(128, NDT * 32)
    x8 = np.ascontiguousarray(x8)

    xt = np.ascontiguousarray(x.T)           # [C, V] fp32

    in_maps = []
    for j in range(N_CORES):
        Sblk = -L[:, VS * j:VS * (j + 1)] * np.float32(SSCALE)
        idx = np.arange(VS)
        Sblk[VS * j + idx, idx] += np.float32(SSCALE)
        Sq = Sblk.astype(e4)                 # [V, 768] fp8
        S8 = Sq[vidx].reshape(NDT, 128, 2 * VS)
        in_maps.append({
            "S8": np.ascontiguousarray(S8),
            "x8": x8,
            "xt": np.ascontiguousarray(xt[:, VS * j:VS * (j + 1)]),
            "ts": ts,
        })
    return in_maps


def kernel(x: np.ndarray, L: np.ndarray, t: np.ndarray) -> np.ndarray:
    global LAST_RESULT
    x = np.ascontiguousarray(np.asarray(x, dtype=np.float32))
    L = np.asarray(L, dtype=np.float32)
    t = np.asarray(t, dtype=np.float32)
    assert x.shape == (V, C) and L.shape == (V, V) and t.shape == (C,)

    in_maps = _host_prep(x, L, t)
    nc = _get_nc()
    res = run_bass_kernel_spmd(nc, in_maps, core_ids=list(range(N_CORES)),
                               trace=TRACE)
    LAST_RESULT = res

    y = np.empty((V, C), dtype=np.float32)
    for j in range(N_CORES):
        y[VS * j:VS * (j + 1), :] = res.results[j]["out"].T
    return x + y
